# revision 27
# baseline (speedup 1.0000x reference)
"""DeepGATEncoder Trainium2 kernel.

8-way data-parallel over the graph axis: each NeuronCore computes one full
graph (2048 nodes, 32768 edges + 2048 self-loops) of the 2-layer GATv2 +
BatchNorm/ELU + TopK-pool encoder, then the MLP head. Host assembles the
[8, 32] output.

Pooling is computed in masked form: dropped nodes are masked (scores forced
to -1e9, gates zeroed) instead of compacted; the final mean over kept nodes
is order-invariant so this matches the reference exactly up to rounding.
Edges from dropped sources are excluded from the layer-2 softmax by writing
per-channel penalty rows (-1e4 * sign(att2)) into dropped nodes' source
features, which drives their logits to -inf regardless of the target
features; dropped-target rows accumulate garbage that the pooling masks.

Segment softmax skips max-subtraction (logits are O(10), exp safe in f32):
out[v] = sum_m p_m x_src(m) / (sum_m p_m + 1e-16).

Segment sums run as dma_scatter_add calls of 256 edges accumulating into
SBUF tables (parity-split CCE mode: node n -> partition n%128, group
(n//128)//2, parity (n//128)%2). Duplicates within a call would race across
SDMA engines, so each call's duplicates are pre-combined on the PE via
equality-matrix matmuls and non-leading duplicates are redirected (host-side
precomputed index stream) to a trash slot. Calls rotate over 2 table pairs
so in-flight calls never touch the same table.
"""
import sys
import os

sys.path.insert(0, '/opt/trn_rl_repo')

import numpy as np

import concourse.bass as bass
import concourse.tile as tile
from concourse import bacc, mybir
from concourse.bass_utils import run_bass_kernel_spmd

F32 = mybir.dt.float32
BF = mybir.dt.bfloat16
FP16 = mybir.dt.float16
I16 = mybir.dt.int16
ALU = mybir.AluOpType
ACTF = mybir.ActivationFunctionType
AX = mybir.AxisListType

B, N, E = 8, 2048, 32768
IN_DIM, HID, LAT = 2, 64, 32
H1, H2 = 4, 2
K1, K2 = 1024, 205
D1, D2 = HID * H1, HID * H2          # 256, 128

EE = E + N                           # 34816 edges incl self-loops
P = 128
CHUNK = 1024                         # edges per pipeline chunk
SLOTS = CHUNK // P                   # 8 tiles of 128 edges per chunk
NCHUNK = EE // CHUNK                 # 34
NCALL = SLOTS // 2                   # 4 scatter calls per chunk
NGRP = 9                             # SBUF table groups (8 node blocks + trash)
TRASH = N                            # scatter idx for dropped duplicates

W1E = D1 + H1                        # 260: GAT1 scatter payload elems
W2E = D2 + H2                        # 130


def _build():
    nc = bacc.Bacc("TRN2", target_bir_lowering=False, debug=False,
                   dynamic_dma_scratch_size=65536, num_swdge_queues=1)

    def din(name, shape, dt=F32):
        return nc.dram_tensor(name, shape, dt, kind="ExternalInput")

    xT_d = din("xT", [IN_DIM, N])
    srcw_d = din("srcw", [P, EE // 16], I16)
    dstw_d = din("dstw", [P, EE // 16], I16)
    ixw_d = din("ixw", [P, EE // 16], I16)
    dstv_d = din("dstv", [P, EE // P], FP16)
    id_d = din("idc", [P, P])
    sel_d = din("selc", [16, 16 * P])
    att1_d = din("att1r", [1, D1])
    att2_d = din("att2r", [1, D2])
    pen_d = din("penr", [1, D2])
    wl1_d = din("wl1", [IN_DIM, D1])
    wr1_d = din("wr1", [IN_DIM, D1])
    bl1_d = din("bl1r", [1, D1])
    br1_d = din("br1r", [1, D1])
    s1_d = din("s1r", [1, D1])
    t1_d = din("t1r", [1, D1])
    wl2_d = din("wl2", [D1, D2])
    wr2_d = din("wr2", [D1, D2])
    bl2_d = din("bl2r", [1, D2])
    br2_d = din("br2r", [1, D2])
    s2_d = din("s2r", [1, D2])
    t2_d = din("t2r", [1, D2])
    pw1_d = din("pw1c", [P, 2])
    pw2_d = din("pw2c", [P, 1])
    w3_d = din("w3", [D2, HID])
    b3_d = din("b3r", [1, HID])
    w4_d = din("w4", [HID, LAT])
    b4_d = din("b4r", [1, LAT])

    out_d = nc.dram_tensor("out", [1, LAT], F32, kind="ExternalOutput")

    xt1 = nc.dram_tensor("xt1", [N, 2 * D1], BF, kind="Internal")
    xt2 = nc.dram_tensor("xt2", [N, 2 * D2], BF, kind="Internal")
    eqsc = nc.dram_tensor("eqsc", [NCHUNK, P, 3 * SLOTS * P // 2], BF, kind="Internal")
    scsc = nc.dram_tensor("scsc", [N], F32, kind="Internal")
    h3sc = nc.dram_tensor("h3sc", [HID], F32, kind="Internal")

    with tile.TileContext(nc) as tc:
        cpool = tc.alloc_tile_pool(name="const", bufs=1)
        tfp = tc.alloc_tile_pool(name="tf", bufs=2)
        ckp = tc.alloc_tile_pool(name="chunk", bufs=2)
        sqp = tc.alloc_tile_pool(name="sq", bufs=2)
        scrp = tc.alloc_tile_pool(name="scr", bufs=1)
        csp = tc.alloc_tile_pool(name="callsrc", bufs=2)
        big = tc.alloc_tile_pool(name="bigpersist", bufs=1)
        ps_rep = tc.alloc_tile_pool(name="ps_rep", bufs=2, space="PSUM")
        ps_tp = tc.alloc_tile_pool(name="ps_tp", bufs=2, space="PSUM")
        ps_call = tc.alloc_tile_pool(name="ps_call", bufs=2, space="PSUM")
        ps_sc = ps_tp

        def cload(dram, shape=None, bcast=False, dt=F32):
            t = cpool.tile(shape or list(dram.shape), dt, tag=f"c_{dram.name}")
            src = dram[:]
            if bcast:
                src = src.to_broadcast(tuple(shape))
            nc.sync.dma_start(t[:], src)
            return t

        ident = cload(id_d)
        identh = cpool.tile([P, P], FP16, tag="c_identh")
        nc.vector.tensor_copy(identh[:], ident[:])
        sel = cload(sel_d)
        selh = cpool.tile([SLOTS, SLOTS * P], FP16, tag="c_selh")
        nc.vector.tensor_copy(selh[:], sel[0:SLOTS, 0:SLOTS * P])
        att1r = cload(att1_d, [P, D1], bcast=True)
        att2r = cload(att2_d, [P, D2], bcast=True)
        penr = cload(pen_d, [P, D2], bcast=True)
        wl1 = cload(wl1_d)
        wr1 = cload(wr1_d)
        bl1r = cload(bl1_d, [P, D1], bcast=True)
        br1r = cload(br1_d, [P, D1], bcast=True)
        s1r = cload(s1_d, [P, D1], bcast=True)
        t1r = cload(t1_d, [P, D1], bcast=True)
        wl2 = cpool.tile([P, 2, D2], F32, tag="c_wl2")
        nc.sync.dma_start(wl2[:], wl2_d[:].rearrange("(t p) c -> p t c", t=2))
        wr2 = cpool.tile([P, 2, D2], F32, tag="c_wr2")
        nc.sync.dma_start(wr2[:], wr2_d[:].rearrange("(t p) c -> p t c", t=2))
        bl2r = cload(bl2_d, [P, D2], bcast=True)
        br2r = cload(br2_d, [P, D2], bcast=True)
        s2r = cload(s2_d, [P, D2], bcast=True)
        t2r = cload(t2_d, [P, D2], bcast=True)
        pw1c = cload(pw1_d)
        pw2c = cload(pw2_d)
        w3s = cload(w3_d)
        b3r = cload(b3_d)
        w4s = cload(w4_d)
        b4r = cload(b4_d)
        srcw = cload(srcw_d, dt=I16)
        dstw = cload(dstw_d, dt=I16)
        ixws = cload(ixw_d, dt=I16)
        dstv = cload(dstv_d, dt=FP16)

        # SBUF accumulation tables: 2 rotating pairs per layer, each pair =
        # (even-parity buf, odd-parity buf) of [P, NGRP, wE] f32. Layer 2
        # aliases layer 1's buffers (same tags); its memsets run after
        # post_layer(1) has consumed the layer-1 sums.
        tabs = {}
        for layer, wE in ((1, W1E), (2, W2E)):
            tl = []
            for k in range(2):
                own = big.tile([P, NGRP, wE], F32, tag=f"tabo{k}")
                oth = big.tile([P, NGRP, wE], F32, tag=f"tabe{k}")
                tl.append((own, oth))
            tabs[layer] = tl

        def zero_tabs(layer):
            for own, oth in tabs[layer]:
                nc.vector.memset(own[:], 0.0)
                nc.vector.memset(oth[:], 0.0)

        zero_tabs(1)

        HSL = SLOTS // 2                       # 4 slots per rr half
        # ---------------- stage 0: equality-matrix prepass (from dst only)
        for c in range(NCHUNK):
            dsl = dstv[:, c * SLOTS:(c + 1) * SLOTS]
            dT_ps = ps_tp.tile([16, P], FP16, space="PSUM", tag="tpose")
            nc.tensor.transpose(dT_ps[:SLOTS, :], dsl[:], identh[:])
            dT = sqp.tile([16, P], FP16, tag="dT_sb")
            nc.scalar.copy(dT[:SLOTS, :], dT_ps[:SLOTS, :])
            eqall = scrp.tile([P, 3 * SLOTS // 2, P], BF, tag="pp_eqall")
            eqa = eqall[:, 0:SLOTS, :]
            eqx = eqall[:, SLOTS:SLOTS + SLOTS // 2, :]
            for hf in range(2):
                rr_ps = ps_rep.tile([P, HSL * P], F32, space="PSUM", tag="rep")
                for t in range(HSL):
                    tt = hf * HSL + t
                    nc.tensor.matmul(rr_ps[:, t * P:(t + 1) * P],
                                     selh[:, tt * P:(tt + 1) * P], dT[:SLOTS, :],
                                     start=True, stop=True)
                rrh = sqp.tile([P, HSL, P], FP16, tag="rrh")
                nc.scalar.copy(rrh[:], rr_ps[:].rearrange("p (s j) -> p s j", s=HSL))
                dslh = dsl[:, hf * HSL:(hf + 1) * HSL]
                nc.vector.tensor_tensor(
                    eqa[:, hf * HSL:(hf + 1) * HSL, :],
                    dslh.unsqueeze(2).broadcast_to((P, HSL, P)),
                    rrh[:], ALU.is_equal)
                nc.vector.tensor_tensor(
                    eqx[:, hf * (HSL // 2):(hf + 1) * (HSL // 2), :],
                    dslh.rearrange("p (a b) -> p a b", b=2)[:, :, 1:2].broadcast_to((P, HSL // 2, P)),
                    rrh[:].rearrange("p (a b) j -> p a b j", b=2)[:, :, 0, :],
                    ALU.is_equal)
            nc.sync.dma_start(eqsc[c, :, :], eqall[:])

        # ---------------- stage 1: GAT1 node transforms -> combined DRAM table
        for g in range(8):
            sb = tfp.tile([P, 2, 2 * D1], BF, tag="tfsb1")
            for h in range(2):
                nt = 2 * g + h
                xg = tfp.tile([IN_DIM, P], F32, tag="xtg")
                nc.sync.dma_start(xg[:], xT_d[:, nt * P:(nt + 1) * P])
                for off, wsb, bsb in ((0, wl1, bl1r), (D1, wr1, br1r)):
                    ps = ps_tp.tile([P, D1], F32, space="PSUM", tag="tpose")
                    nc.tensor.matmul(ps[:], xg[:], wsb[:], start=True, stop=True)
                    nc.vector.tensor_tensor(sb[:, h, off:off + D1], ps[:], bsb[:], ALU.add)
            nc.sync.dma_start(xt1[2 * P * g:2 * P * (g + 1), :].rearrange("(b p) c -> p b c", b=2),
                              sb[:])

        # ---------------- edge pipeline (both layers)
        def edge_layer(layer):
            if layer == 1:
                tab, elem, heads, attr = xt1, D1, H1, att1r
                wE = W1E
            else:
                tab, elem, heads, attr = xt2, D2, H2, att2r
                wE = W2E
            own = [tabs[layer][0][0], tabs[layer][1][0]]
            oth = [tabs[layer][0][1], tabs[layer][1][1]]
            for c in range(NCHUNK):
                i0 = c * CHUNK
                xj = ckp.tile([P, SLOTS, elem], BF, tag="xj")
                xi = ckp.tile([P, SLOTS, elem], BF, tag="xi")
                nc.gpsimd.dma_gather(
                    out_ap=xj[:], in_ap=tab[:, 0:elem],
                    idxs_ap=srcw[:, i0 // 16:(i0 + CHUNK) // 16],
                    num_idxs=CHUNK, num_idxs_reg=CHUNK, elem_size=elem,
                    elem_step=2 * elem)
                nc.gpsimd.dma_gather(
                    out_ap=xi[:], in_ap=tab[:, elem:2 * elem],
                    idxs_ap=dstw[:, i0 // 16:(i0 + CHUNK) // 16],
                    num_idxs=CHUNK, num_idxs_reg=CHUNK, elem_size=elem,
                    elem_step=2 * elem)
                nc.vector.tensor_tensor(xi[:], xj[:], xi[:], ALU.add)
                nc.vector.scalar_tensor_tensor(xi[:], xi[:], 0.2, xi[:],
                                               op0=ALU.mult, op1=ALU.max)
                nc.vector.tensor_tensor(
                    xi[:], xi[:],
                    attr[:, 0:elem].unsqueeze(1).broadcast_to((P, SLOTS, elem)), ALU.mult)
                red = sqp.tile([P, SLOTS, heads], F32, tag=f"red{layer}")
                nc.vector.tensor_reduce(
                    red[:], xi[:].rearrange("p s (h k) -> p s h k", h=heads),
                    AX.X, ALU.add)
                pp = sqp.tile([P, SLOTS, heads], F32, tag=f"pp{layer}")
                nc.scalar.activation(pp[:].rearrange("p s h -> p (s h)"),
                                     red[:].rearrange("p s h -> p (s h)"), ACTF.Exp)
                wm = ckp.tile([P, SLOTS, wE], BF, tag="wm")
                nc.vector.tensor_tensor(
                    wm[:, :, 0:elem].rearrange("p s (h k) -> p s h k", h=heads),
                    xj[:].rearrange("p s (h k) -> p s h k", h=heads),
                    pp[:].unsqueeze(3).broadcast_to((P, SLOTS, heads, 64)),
                    ALU.mult)
                nc.scalar.copy(wm[:, :, elem:wE], pp[:])

                eqt = ckp.tile([P, 3 * SLOTS // 2, P], BF, tag="eqt")
                nc.sync.dma_start(eqt[:], eqsc[c, :, :].rearrange("p (s j) -> p s j", j=P))
                eqa = eqt[:, 0:SLOTS, :]
                eqx = eqt[:, SLOTS:SLOTS + SLOTS // 2, :]
                for i in range(NCALL):
                    gc = c * NCALL + i
                    k = gc % 2
                    pa = ps_call.tile([P, wE], F32, space="PSUM", tag="call")
                    nc.tensor.matmul(pa[:], eqa[:, 2 * i, :], wm[:, 2 * i, :],
                                     start=True, stop=False)
                    nc.tensor.matmul(pa[:], eqx[:, i, :], wm[:, 2 * i + 1, :],
                                     start=False, stop=True)
                    pb = ps_call.tile([P, wE], F32, space="PSUM", tag="call")
                    nc.tensor.matmul(pb[:], eqa[:, 2 * i + 1, :], wm[:, 2 * i + 1, :],
                                     start=True, stop=True)
                    cs = csp.tile([P, 2, wE], F32, tag="cs")
                    nc.scalar.copy(cs[:, 0, :], pa[:])
                    nc.scalar.copy(cs[:, 1, :], pb[:])
                    nc.gpsimd.dma_scatter_add(
                        out_ap=own[k][:],
                        in_ap=cs[:],
                        idxs_ap=ixws[:, i0 // 16 + i * 16:i0 // 16 + (i + 1) * 16],
                        num_idxs=2 * P,
                        num_idxs_reg=2 * P,
                        elem_size=wE,
                        queue_num=0,
                        sbuf_tokens_per_rank=P,
                        parity_reg=0,
                        out_ap_other=oth[k][:])

        # ---------------- shared post helpers
        def post_layer(layer, hc):
            wE = W1E if layer == 1 else W2E
            ddim = D1 if layer == 1 else D2
            heads = H1 if layer == 1 else H2
            srow = s1r if layer == 1 else s2r
            trow = t1r if layer == 1 else t2r
            (o0, e0), (o1, e1) = tabs[layer]
            for nt in range(16):
                par, g = nt % 2, nt // 2
                a = o0 if par == 0 else e0
                b = o1 if par == 0 else e1
                acc = tfp.tile([P, wE], F32, tag="acc")
                nc.vector.tensor_tensor(acc[:], a[:, g, :], b[:, g, :], ALU.add)
                dn = tfp.tile([P, heads], F32, tag="dn")
                nc.vector.tensor_scalar(dn[:], acc[:, ddim:wE], 1e-16, None, ALU.add)
                nc.vector.reciprocal(dn[:], dn[:])
                y = tfp.tile([P, ddim], F32, tag="y")
                for hd in range(heads):
                    nc.vector.tensor_scalar(
                        y[:, hd * 64:(hd + 1) * 64], acc[:, hd * 64:(hd + 1) * 64],
                        dn[:, hd:hd + 1], None, ALU.mult)
                nc.vector.tensor_tensor(y[:], y[:], srow[:, 0:ddim], ALU.mult)
                nc.vector.tensor_tensor(y[:], y[:], trow[:, 0:ddim], ALU.add)
                m = tfp.tile([P, ddim], F32, tag="melu")
                nc.vector.tensor_scalar(m[:], y[:], 0.0, None, ALU.min)
                nc.scalar.activation(m[:], m[:], ACTF.Exp)
                nc.vector.tensor_scalar(m[:], m[:], -1.0, None, ALU.add)
                nc.vector.tensor_tensor(y[:], y[:], m[:], ALU.max)
                for hb in range(ddim // P):
                    tp = ps_tp.tile([P, P], F32, space="PSUM", tag="tpose")
                    nc.tensor.transpose(tp[:], y[:, hb * P:(hb + 1) * P], ident[:])
                    nc.scalar.copy(hc[:, hb, nt * P:(nt + 1) * P], tp[:])

        def score_cols(hc, nchtiles, pwc):
            for q in range(4):
                scp = ps_sc.tile([1, 512], F32, space="PSUM", tag="tpose")
                for ct in range(nchtiles):
                    nc.tensor.matmul(scp[:], pwc[:, ct:ct + 1], hc[:, ct, q * 512:(q + 1) * 512],
                                     start=(ct == 0), stop=(ct == nchtiles - 1))
                scrow = scrp.tile([1, 512], F32, tag="scrow")
                nc.scalar.copy(scrow[:], scp[:])
                nc.sync.dma_start(scsc[q * 512:(q + 1) * 512], scrow[:])
            sccol = sqp.tile([P, 16], F32, tag="sccol")
            nc.sync.dma_start(sccol[:], scsc[:].rearrange("(f p) -> p f", p=P))
            return sccol

        def rowrep_half(colt, half):
            """[128,16] column layout -> [128, 1024] row-replicated PSUM for tiles half*8..half*8+8."""
            cT_ps = ps_tp.tile([16, P], F32, space="PSUM", tag="tpose")
            nc.tensor.transpose(cT_ps[:], colt[:], ident[:])
            cT = sqp.tile([16, P], F32, tag="rrT_sb")
            nc.scalar.copy(cT[:], cT_ps[:])
            rep_ps = ps_rep.tile([P, 8 * P], F32, space="PSUM", tag="rep")
            for t in range(8):
                tt = half * 8 + t
                nc.tensor.matmul(rep_ps[:, t * P:(t + 1) * P], sel[:, tt * P:(tt + 1) * P], cT[:],
                                 start=True, stop=True)
            return rep_ps

        def rank_keep2(colt, kk):
            ranka = sqp.tile([P, 16], F32, tag="ranka")
            rankb = sqp.tile([P, 16], F32, tag="rankb")
            scratch = scrp.tile([P, 8 * P], F32, tag="rkscr")
            for half, rk in ((0, ranka), (1, rankb)):
                rep = rowrep_half(colt, half)
                for f in range(16):
                    nc.vector.tensor_scalar(
                        scratch[:], rep[:], colt[:, f:f + 1], None, ALU.is_gt, ALU.add,
                        accum_out=rk[:, f:f + 1])
            nc.vector.tensor_tensor(ranka[:], ranka[:], rankb[:], ALU.add)
            keep = sqp.tile([P, 16], F32, tag="keep")
            nc.vector.tensor_scalar(keep[:], ranka[:], float(kk), None, ALU.is_lt)
            return keep

        # ================ main flow
        REPEAT = int(os.environ.get("GAT_REPEAT", "1"))
        for _rep in range(REPEAT - 1):
            edge_layer(1)
            edge_layer(2)
        edge_layer(1)

        h1c = big.tile([P, 2, N], F32, tag="h1c")
        post_layer(1, h1c)

        sc1col = score_cols(h1c, 2, pw1c)
        keep1 = big.tile([P, 16], F32)
        nc.vector.tensor_copy(keep1[:], rank_keep2(sc1col, K1)[:])
        gate1 = sqp.tile([P, 16], F32, tag="gate1")
        nc.scalar.activation(gate1[:], sc1col[:], ACTF.Tanh)
        nc.vector.tensor_tensor(gate1[:], gate1[:], keep1[:], ALU.mult)
        for half in range(2):
            g1rep = rowrep_half(gate1, half)
            for hb in range(2):
                nc.vector.tensor_tensor(
                    h1c[:, hb, half * 1024:(half + 1) * 1024],
                    h1c[:, hb, half * 1024:(half + 1) * 1024],
                    g1rep[:], ALU.mult)

        invk = big.tile([P, 16], F32)
        nc.vector.tensor_scalar(invk[:], keep1[:], -1.0, -1.0, ALU.add, ALU.mult)

        # ---------------- GAT2 node transforms -> combined table with
        # penalty rows for dropped sources
        for g in range(4):
            sb = tfp.tile([P, 4, 2 * D2], BF, tag="tfsb1")
            for h in range(4):
                nt = 4 * g + h
                psl = ps_tp.tile([P, D2], F32, space="PSUM", tag="tpose")
                for ct in range(2):
                    nc.tensor.matmul(psl[:], h1c[:, ct, nt * P:(nt + 1) * P], wl2[:, ct, :],
                                     start=(ct == 0), stop=(ct == 1))
                yl = tfp.tile([P, D2], F32, tag="tf2yl")
                nc.vector.tensor_tensor(yl[:], psl[:], bl2r[:], ALU.add)
                nc.vector.tensor_scalar(yl[:], yl[:], keep1[:, nt:nt + 1], None, ALU.mult)
                pen = tfp.tile([P, D2], F32, tag="tf2pen")
                nc.vector.tensor_scalar(pen[:], penr[:], invk[:, nt:nt + 1], None, ALU.mult)
                nc.vector.tensor_tensor(sb[:, h, 0:D2], yl[:], pen[:], ALU.add)
                psr = ps_tp.tile([P, D2], F32, space="PSUM", tag="tpose")
                for ct in range(2):
                    nc.tensor.matmul(psr[:], h1c[:, ct, nt * P:(nt + 1) * P], wr2[:, ct, :],
                                     start=(ct == 0), stop=(ct == 1))
                nc.vector.tensor_tensor(sb[:, h, D2:2 * D2], psr[:], br2r[:], ALU.add)
            nc.sync.dma_start(xt2[4 * P * g:4 * P * (g + 1), :].rearrange("(b p) c -> p b c", b=4),
                              sb[:])

        zero_tabs(2)
        edge_layer(2)

        h2c = big.tile([P, 1, N], F32, tag="h1c")
        post_layer(2, h2c)

        sc2col = score_cols(h2c, 1, pw2c)
        t9 = sqp.tile([P, 16], F32, tag="t9")
        nc.vector.tensor_scalar(t9[:], keep1[:], -1.0, 1e9, ALU.add, ALU.mult)
        sc2m = sqp.tile([P, 16], F32, tag="sc2m")
        nc.vector.tensor_tensor(sc2m[:], sc2col[:], keep1[:], ALU.mult)
        nc.vector.tensor_tensor(sc2m[:], sc2m[:], t9[:], ALU.add)
        keep2 = rank_keep2(sc2m, K2)
        gate2 = sqp.tile([P, 16], F32, tag="gate2")
        nc.scalar.activation(gate2[:], sc2col[:], ACTF.Tanh)
        nc.vector.tensor_tensor(gate2[:], gate2[:], keep2[:], ALU.mult)
        for half in range(2):
            g2rep = rowrep_half(gate2, half)
            nc.vector.tensor_tensor(
                h2c[:, 0, half * 1024:(half + 1) * 1024],
                h2c[:, 0, half * 1024:(half + 1) * 1024],
                g2rep[:], ALU.mult)
        pooled = sqp.tile([P, 1], F32, tag="pooled")
        nc.vector.tensor_reduce(pooled[:], h2c[:, 0, :], AX.X, ALU.add)
        nc.vector.tensor_scalar(pooled[:], pooled[:], 1.0 / K2, None, ALU.mult)
        ps3 = ps_sc.tile([1, 512], F32, space="PSUM", tag="tpose")
        nc.tensor.matmul(ps3[:, 0:HID], pooled[:], w3s[:], start=True, stop=True)
        h3 = sqp.tile([1, HID], F32, tag="h3")
        nc.vector.tensor_tensor(h3[:], ps3[:, 0:HID], b3r[:], ALU.add)
        nc.vector.tensor_scalar(h3[:], h3[:], 0.0, None, ALU.max)
        nc.sync.dma_start(h3sc[:], h3[:])
        h3c = sqp.tile([HID, 1], F32, tag="h3c")
        nc.sync.dma_start(h3c[:], h3sc[:].rearrange("(a b) -> a b", b=1))
        ps4 = ps_sc.tile([1, 512], F32, space="PSUM", tag="tpose")
        nc.tensor.matmul(ps4[:, 0:LAT], h3c[:], w4s[:], start=True, stop=True)
        ot = sqp.tile([1, LAT], F32, tag="ot")
        nc.vector.tensor_tensor(ot[:], ps4[:, 0:LAT], b4r[:], ALU.add)
        nc.sync.dma_start(out_d[:], ot[:])

        for p_ in (ps_call, ps_tp, ps_rep, big, csp, scrp, sqp, ckp, tfp, cpool):
            p_.release()

    nc.compile()
    return nc


def _wrap16(v):
    """[EE] stream -> [128, EE//16] scatter/gather idx layout: position j at
    [j%16, j//16], replicated over the 8 gpsimd cores' partition groups."""
    w = np.ascontiguousarray(v.reshape(-1, 16).T)
    return np.tile(w, (8, 1))


def _make_inputs(inputs):
    x = np.asarray(inputs["x"], np.float32)
    src = np.asarray(inputs["src"], np.int32)
    dst = np.asarray(inputs["dst"], np.int32)

    def row(v):
        return np.ascontiguousarray(np.asarray(v, np.float32).reshape(1, -1))

    eps = 1e-5
    g1 = np.asarray(inputs["g1"], np.float32)
    s1 = g1 / np.sqrt(np.asarray(inputs["var1"], np.float32) + eps)
    t1 = np.asarray(inputs["bias1"], np.float32) * s1 + (
        np.asarray(inputs["be1"], np.float32) - np.asarray(inputs["mu1"], np.float32) * s1)
    g2 = np.asarray(inputs["g2"], np.float32)
    s2 = g2 / np.sqrt(np.asarray(inputs["var2"], np.float32) + eps)
    t2 = np.asarray(inputs["bias2"], np.float32) * s2 + (
        np.asarray(inputs["be2"], np.float32) - np.asarray(inputs["mu2"], np.float32) * s2)
    att1 = np.asarray(inputs["att1"], np.float32).reshape(1, -1)
    att2 = np.asarray(inputs["att2"], np.float32).reshape(1, -1)
    pen = -1e4 * np.sign(att2)
    pw1 = np.asarray(inputs["pw1"], np.float32)
    pw1n = np.ascontiguousarray((pw1 / (np.linalg.norm(pw1) + 1e-16)).reshape(2, P).T)
    pw2 = np.asarray(inputs["pw2"], np.float32)
    pw2n = np.ascontiguousarray((pw2 / (np.linalg.norm(pw2) + 1e-16)).reshape(1, P).T)

    sel = np.zeros((16, 16 * P), np.float32)
    for t in range(16):
        sel[t, t * P:(t + 1) * P] = 1.0

    shared = {
        "idc": np.eye(P, dtype=np.float32),
        "selc": sel,
        "att1r": att1,
        "att2r": att2,
        "penr": pen,
        "wl1": np.asarray(inputs["Wl1"], np.float32),
        "wr1": np.asarray(inputs["Wr1"], np.float32),
        "bl1r": row(inputs["bl1"]),
        "br1r": row(inputs["br1"]),
        "s1r": row(s1), "t1r": row(t1),
        "wl2": np.asarray(inputs["Wl2"], np.float32),
        "wr2": np.asarray(inputs["Wr2"], np.float32),
        "bl2r": row(inputs["bl2"]),
        "br2r": row(inputs["br2"]),
        "s2r": row(s2), "t2r": row(t2),
        "pw1c": pw1n, "pw2c": pw2n,
        "w3": np.asarray(inputs["W3"], np.float32),
        "b3r": row(inputs["b3"]),
        "w4": np.asarray(inputs["W4"], np.float32),
        "b4r": row(inputs["b4"]),
    }

    loops = np.arange(N, dtype=np.int32)
    in_maps = []
    for b in range(B):
        es = np.concatenate([src[b], loops])
        ed = np.concatenate([dst[b], loops])
        # per-256-edge-window dedup: non-leading duplicate targets -> trash
        W = ed.reshape(-1, 2 * P)
        order = np.argsort(W, axis=1, kind="stable")
        sv = np.take_along_axis(W, order, axis=1)
        dup_s = np.zeros_like(sv, bool)
        dup_s[:, 1:] = sv[:, 1:] == sv[:, :-1]
        dup = np.zeros_like(dup_s)
        np.put_along_axis(dup, order, dup_s, axis=1)
        ix = np.where(dup, TRASH, W).reshape(-1)
        m = dict(shared)
        m["xT"] = np.ascontiguousarray(x[b].T)
        m["srcw"] = _wrap16(es.astype(np.int16))
        m["dstw"] = _wrap16(ed.astype(np.int16))
        m["ixw"] = _wrap16(ix.astype(np.int16))
        dv = np.zeros((P, EE // P), np.float16)
        k = np.arange(EE)
        dv[k % P, k // P] = ed.astype(np.float16)
        m["dstv"] = dv
        in_maps.append(m)
    return in_maps


_CACHE = {}


def kernel(**inputs):
    if "nc" not in _CACHE:
        _CACHE["nc"] = _build()
    nc = _CACHE["nc"]
    in_maps = _make_inputs(inputs)
    res = run_bass_kernel_spmd(nc, in_maps, core_ids=list(range(B)),
                               trace=bool(int(os.environ.get("GAT_TRACE", "0"))))
    _CACHE["last"] = res
    out = np.concatenate([res.results[i]["out"] for i in range(B)], axis=0)
    return out.astype(np.float32)
